# revision 14
# baseline (speedup 1.0000x reference)
"""Trainium2 Bass kernel for a 3-layer GraphSAGE-GCN (gnn_message_passing).

Math (per layer, commuting the dense matmul through the linear aggregation):
    Y_l   = h_{l-1} @ W_l^T                      (dense per-node matmul)
    h_l   = relu(inv ⊙ (A + I) Y_l)              (edge gather + scatter-add)
with inv = 1/(deg_in + 1) (self-loops via an identity matmul on the PE).

Distribution: destination nodes (and their incoming edges) are sharded
across 8 NeuronCores; the small [6250, D] per-core matmul outputs are
AllGather'ed into a full [50000, D] gather source between layers.

Scatter-add on device: destination nodes are grouped in blocks of
BP=112; edges are host-bucketed by (dst block, src half) into cells.
One dma_gather call per cell pulls Y[src] rows into SBUF (<= 8 chunks
of 128 edges = 1024 descriptors, the SWDGE ring capacity — more hangs
the DGE). A one-hot selection matrix built on DVE in the interleaved
layout o[p, j*CTOT+k] = (j == dst_slot[p, k]) — which keeps every
operand's last dim stride-1 so the 2x 16-bit DVE mode applies — drives
PE matmuls O^T @ G that accumulate the per-dst-block aggregate in PSUM.
The self term is one more PE matmul (identity x y_self), and the
scale+relu runs on the ACT engine straight out of PSUM.

Gather padding is skipped at run time: pad slots carry index -1 and
each call's true descriptor count is read per-core from an input table
into a Pool register (num_idxs_reg), so the DMA engines and the SWDGE
descriptor generator only touch real edges. Gather tile buffers are
memset once in the prologue so slots beyond the count hold finite data
(the one-hot kills them, but 0 * non-finite would poison the PE).
The src half split exists because dma_gather indices are int16.
"""

import math

import numpy as np

import concourse.bacc as bacc
import concourse.bass as bass
import concourse.mybir as mybir
import concourse.tile as tile
from concourse.bass_utils import run_bass_kernel_spmd
from concourse.masks import make_identity

# Problem constants (hardcoded per harness contract).
N = 50000
DIN = 128
DH = 128
DZ = 64
NCORES = 8
P = 128                      # partitions / gather chunk size
BP = 112                     # dst nodes per block (cell <= 1024 edges)
NLOC = N // NCORES           # 6250 destination nodes per core
NBLK = (NLOC + BP - 1) // BP  # 56 dst blocks per core
HALF = 25000                 # src index split so gather indices fit int16

F32 = mybir.dt.float32
FP16 = mybir.dt.float16
I16 = mybir.dt.int16
U32 = mybir.dt.uint32
ACT = mybir.ActivationFunctionType
BUILD_STAGES = 6
NQUEUES = 1
GBUFS = 12
SCRATCH = 16384
PF_DIST = 3
OH_DIST = 1
OPOOL_BUFS = 3
HPOOL_BUFS = 6
AGG_BUFS = 4
MOCK_COLLECTIVES = False
SKIP_GATHER = False
SKIP_MM = False
LIMIT_BLOCKS = None
REPEATS = 1
FULL_COUNTS = False
IMM_COUNTS = False


def _cdiv(a, b):
    return (a + b - 1) // b


def _preprocess(x, edge_index):
    """Host-side graph partitioning: bucket edges by (core, dst block, src
    half), build the int16 gather index arrays (pad = -1), per-call true
    descriptor counts, the one-hot dst-slot arrays, degree counts and
    transposed node features."""
    src = np.asarray(edge_index[0], dtype=np.int64)
    dst = np.asarray(edge_index[1], dtype=np.int64)
    cnt = np.bincount(dst, minlength=N).astype(np.float32) + 1.0  # deg + 1

    core = dst // NLOC
    ldst = dst - core * NLOC
    blk = ldst // BP
    slot = ldst % BP
    half = (src >= HALF).astype(np.int64)
    ncells = NCORES * NBLK * 2
    cell = (core * 2 + half) * NBLK + blk  # half-major: lo cells, then hi

    order = np.argsort(cell, kind="stable")
    cell_s = cell[order]
    half_s = half[order]
    vals = (src[order] - half_s * HALF).astype(np.int16)
    slot_s = slot[order].astype(np.float32)

    counts = np.bincount(cell_s, minlength=ncells).astype(np.int64)
    by_half = counts.reshape(NCORES, 2, NBLK)
    C_LO = max(1, _cdiv(int(by_half[:, 0, :].max()), P))
    C_HI = max(1, _cdiv(int(by_half[:, 1, :].max()), P))
    CTOT = C_LO + C_HI
    CPC = NBLK * CTOT          # chunks per core
    COLS = CPC * 8             # int16 idx columns per core

    cell_start = np.zeros(ncells + 1, np.int64)
    cell_start[1:] = np.cumsum(counts)
    q = np.arange(len(cell_s)) - cell_start[cell_s]  # rank within own cell

    core_s = cell_s // (NBLK * 2)
    b_s = cell_s % NBLK
    # idx stream layout per core is half-major (lo cells then hi cells) so
    # each (block, half) gather call reads contiguous index columns; the
    # dmod (one-hot) layout is block-major so one broadcast tensor_tensor
    # builds a whole block's selection matrices.
    chunk0_i = np.where(half_s == 0, b_s * C_LO, NBLK * C_LO + b_s * C_HI)
    chunk0_d = b_s * CTOT + half_s * C_LO

    # Pad positions carry idx -1; they are never transferred (the dynamic
    # descriptor count stops before them). Gather tile buffers are zeroed
    # once in the kernel prologue, so trailing slots stay finite.
    idx_arr = np.full((NCORES, 16, COLS), -1, np.int16)
    idx_arr[core_s, q % 16, chunk0_i * 8 + q // 16] = vals
    mod_arr = np.full((NCORES, P, CPC), -1.0, np.float16)
    mod_arr[core_s, q % P, chunk0_d + q // P] = slot_s

    # Per-(core, block, half) true counts; 0-count cells get one dummy
    # valid idx (row 0, killed by the all-(-1) one-hot column) because the
    # descriptor count must equal the number of non-negative indices.
    cnt_cell = by_half.transpose(0, 2, 1)  # [NC, NBLK, 2]
    if FULL_COUNTS:
        # debug/validation mode: gather every slot (pad rows = 0)
        np.copyto(idx_arr, np.where(idx_arr < 0, 0, idx_arr))
        gcnt = np.broadcast_to(
            np.array([C_LO * P, C_HI * P], np.uint32)[None, None, :],
            cnt_cell.shape).copy()
    else:
        gcnt = np.maximum(cnt_cell, 1).astype(np.uint32)
        for c, b, h in zip(*np.nonzero(cnt_cell == 0)):
            base = (b * C_LO if h == 0 else NBLK * C_LO + b * C_HI) * 8
            idx_arr[c, 0, base] = 0
    gcnt = gcnt.reshape(NCORES, NBLK * 2)

    npad = NBLK * BP
    cnt_pad = np.ones((NCORES, npad), np.float32)
    cnt_pad[:, :NLOC] = cnt.reshape(NCORES, NLOC)
    cnt_arr = np.ones((NCORES, P, NBLK), np.float32)
    cnt_arr[:, :BP, :] = cnt_pad.reshape(NCORES, NBLK, BP).transpose(0, 2, 1)

    x = np.asarray(x, dtype=np.float32)
    x_pad = np.zeros((NCORES, npad, DIN), np.float32)
    x_pad[:, :NLOC] = x.reshape(NCORES, NLOC, DIN)
    xT = np.ascontiguousarray(x_pad.transpose(0, 2, 1))  # [NC, DIN, npad]

    # interleaved iota: col j*CTOT + k holds value j (see one-hot layout)
    iota = np.repeat(np.arange(BP, dtype=np.float16), CTOT)[None, :]
    iota = np.ascontiguousarray(np.broadcast_to(iota, (P, BP * CTOT)))

    return dict(C_LO=C_LO, C_HI=C_HI, idx=idx_arr, dmod=mod_arr, cnt=cnt_arr,
                xT=xT, iota=iota, gcnt=gcnt)


def _build(C_LO, C_HI):
    """Build + compile the SPMD Bass program (identical on all cores)."""
    assert C_LO <= 8 and C_HI <= 8, \
        f"cell exceeds the 1024-descriptor SWDGE ring ({C_LO=}, {C_HI=})"
    CTOT = C_LO + C_HI
    CPC = NBLK * CTOT
    COLS = CPC * 8

    nc = bacc.Bacc("TRN2", target_bir_lowering=False, debug=False,
                   num_devices=NCORES, num_swdge_queues=NQUEUES,
                   dynamic_dma_scratch_size=SCRATCH)

    xT_d = nc.dram_tensor("xT", [DIN, NBLK * BP], F32, kind="ExternalInput")
    idx_d = nc.dram_tensor("idx", [16, COLS], I16, kind="ExternalInput")
    dmod_d = nc.dram_tensor("dmod", [P, CPC], FP16, kind="ExternalInput")
    cnt_d = nc.dram_tensor("cnt", [P, NBLK], F32, kind="ExternalInput")
    iota_d = nc.dram_tensor("iota", [P, CTOT * BP], FP16,
                            kind="ExternalInput")
    gcnt_d = nc.dram_tensor("gcnt", [1, NBLK * 2], U32, kind="ExternalInput")
    w1t_d = nc.dram_tensor("w1t", [DIN, DH], F32, kind="ExternalInput")
    w2t_d = nc.dram_tensor("w2t", [DH, DH], F32, kind="ExternalInput")
    w3t_d = nc.dram_tensor("w3t", [DH, DZ], F32, kind="ExternalInput")
    out_d = nc.dram_tensor("out", [NLOC, DZ], F32, kind="ExternalOutput")

    # Y3 rows are padded to 128 fp16 (256B, the dma_gather minimum elem);
    # cols 64:128 are never written or read.
    ydt = [FP16, FP16, FP16]
    y_loc = [nc.dram_tensor(f"y{l}loc", [NLOC, d], ydt[l - 1])
             for l, d in ((1, DH), (2, DH), (3, DH))]
    y_full = [nc.dram_tensor(f"y{l}full", [N, d], ydt[l - 1],
                             addr_space="Shared")
              for l, d in ((1, DH), (2, DH), (3, DH))]

    rows_of = [min(BP, NLOC - b * BP) for b in range(NBLK)]

    with tile.TileContext(nc) as tc:
        with (
            tc.tile_pool(name="pers", bufs=1) as pers,
            tc.tile_pool(name="gpool", bufs=GBUFS) as gpool,
            tc.tile_pool(name="opool", bufs=OPOOL_BUFS) as opool,
            tc.tile_pool(name="hpool", bufs=HPOOL_BUFS) as hpool,
            tc.tile_pool(name="agg_ps", bufs=AGG_BUFS, space="PSUM") as agg_pp,
            tc.tile_pool(name="tr_ps", bufs=8 - AGG_BUFS - 1, space="PSUM") as tr_pp,
            tc.tile_pool(name="y_ps", bufs=1, space="PSUM") as y_pp,
        ):
            # --- persistent tiles -------------------------------------------
            # The Q7 descriptor generators read the index stream through
            # their own 16-partition groups: the indices must be replicated
            # into all eight groups.
            idx_sb = pers.tile([P, COLS], I16)
            for gidx in range(8):
                nc.sync.dma_start(idx_sb[gidx * 16:(gidx + 1) * 16, :],
                                  idx_d[:, :])
            dmod_sb = pers.tile([P, CPC], FP16)
            nc.sync.dma_start(dmod_sb[:], dmod_d[:, :])
            iota_sb = pers.tile([P, CTOT * BP], FP16)
            nc.sync.dma_start(iota_sb[:], iota_d[:, :])
            cnt_sb = pers.tile([P, NBLK], F32)
            nc.sync.dma_start(cnt_sb[:], cnt_d[:, :])
            gcnt_sb = pers.tile([1, NBLK * 2], U32)
            nc.sync.dma_start(gcnt_sb[:], gcnt_d[:, :])
            inv_sb = pers.tile([P, NBLK], F32)
            nc.vector.reciprocal(inv_sb[:], cnt_sb[:])
            w1t_sb = pers.tile([DIN, DH], F32)
            nc.sync.dma_start(w1t_sb[:], w1t_d[:, :])
            w2t_sb = pers.tile([DH, DH], F32)
            nc.sync.dma_start(w2t_sb[:], w2t_d[:, :])
            w3t_sb = pers.tile([DH, DZ], F32)
            nc.sync.dma_start(w3t_sb[:], w3t_d[:, :])
            ident = pers.tile([P, P], F32)
            make_identity(nc, ident[:])
            ident16 = pers.tile([P, P], FP16)
            make_identity(nc, ident16[:])
            xT_sb = pers.tile([DIN, NBLK * BP], F32)
            nc.sync.dma_start(xT_sb[:], xT_d[:, :])

            def allgather(li):
                if MOCK_COLLECTIVES:
                    # timing-only single-core variant (TimelineSim has no
                    # collectives); data correctness not preserved
                    nc.sync.dma_start(y_full[li][0:NLOC, :], y_loc[li][:, :])
                    return
                nc.gpsimd.collective_compute(
                    "AllGather", mybir.AluOpType.bypass,
                    ins=[y_loc[li][:, :]], outs=[y_full[li][:, :]],
                    replica_groups=[list(range(NCORES))])

            call_no = [0]
            cnt_regs = [None]

            def agg_layer(li, D, w_next_sb, D_next):
                EL = DH  # gather elem width (Y3 rows are padded to DH)
                gdt = ydt[li]
                """Aggregate y_full[li] into h, then either compute the next
                layer's Y (w_next_sb) or write the final output.

                One dma_gather call per (block, half) cell; the call's true
                descriptor count comes from the gcnt table (pad idxs are -1
                and are never transferred).
                """
                yf = y_full[li]
                src_ap = [yf[0:HALF, :], yf[HALF:N, :]]
                col0 = [0, NBLK * C_LO * 8]         # idx col base per half
                tiles = [[], []]                    # emitted gather tiles
                gct = gcnt_sb
                cnt_regs[0] = cnt_regs[0] or [
                    nc.gpsimd.alloc_register(f"gcr{q}")
                    for q in range(NQUEUES)]

                def ensure_call(h, b):
                    # Emit gather calls for half h up to block b.
                    while len(tiles[h]) <= b:
                        bb = len(tiles[h])
                        nch = (C_LO, C_HI)[h]
                        g = gpool.tile([P, nch * EL], gdt, tag=f"g{h}")
                        c0 = col0[h] + bb * nch * 8
                        if not SKIP_GATHER:
                            # One count register per queue, redefined per
                            # call; the scheduler tracks register deps and
                            # same-queue calls serialize on the SWDGE ring.
                            q = call_no[0] % NQUEUES
                            if IMM_COUNTS:
                                cntv = nch * P
                            else:
                                cntv = cnt_regs[0][q]
                                nc.gpsimd.reg_load(
                                    cntv,
                                    gct[0:1, bb * 2 + h:bb * 2 + h + 1])
                            nc.gpsimd.dma_gather(
                                g[:].rearrange("p (c d) -> p c d", d=EL),
                                src_ap[h], idx_sb[:, c0:c0 + nch * 8],
                                nch * P, cntv, EL,
                                queue_num=q)
                        else:
                            nc.vector.memset(g[:, :P], 0)
                        call_no[0] += 1
                        tiles[h].append(g)
                    return tiles[h][b]

                nblk = NBLK if LIMIT_BLOCKS is None else LIMIT_BLOCKS

                def emit_oh(b):
                    # one broadcast is_equal builds all CTOT selection
                    # matrices of block b in the interleaved layout
                    # oh[p, j*CTOT + k] = (j == dmod[p, b*CTOT + k]);
                    # every operand's last dim is stride-1 so the 2x
                    # 16-bit DVE mode applies.
                    oh = opool.tile([P, CTOT * BP], gdt, tag="oh")
                    dm = dmod_sb[:, b * CTOT:(b + 1) * CTOT]
                    nc.vector.tensor_tensor(
                        out=oh[:].rearrange("p (j k) -> p j k", k=CTOT),
                        in0=iota_sb[:].rearrange("p (j k) -> p j k", k=CTOT),
                        in1=dm.unsqueeze(1).broadcast_to([P, BP, CTOT]),
                        op=mybir.AluOpType.is_equal)
                    return oh

                def emit_tail(b, agg):
                    r = rows_of[b]
                    h_sb = hpool.tile([P, D], F32, tag="hsb")
                    nc.scalar.activation(
                        h_sb[:BP, :], agg[:BP, :], ACT.Relu,
                        scale=inv_sb[:BP, b:b + 1])
                    if w_next_sb is None:
                        nc.sync.dma_start(out_d[b * BP:b * BP + r, :],
                                          h_sb[:r, :])
                    else:
                        hT_ps = tr_pp.tile([P, P], F32, tag="htps")
                        nc.tensor.transpose(hT_ps[:, :BP], h_sb[:BP, :],
                                            ident[:BP, :BP])
                        hT_sb = hpool.tile([P, P], F32, tag="htsb")
                        nc.scalar.activation(hT_sb[:, :BP], hT_ps[:, :BP],
                                             ACT.Copy)
                        y_ps = y_pp.tile([P, D_next], F32, tag="yps")
                        nc.tensor.matmul(y_ps[:BP, :], lhsT=hT_sb[:, :BP],
                                         rhs=w_next_sb[:], start=True,
                                         stop=True)
                        y_sb = hpool.tile([P, D_next], ydt[li + 1],
                                          tag="ysb")
                        nc.scalar.activation(y_sb[:BP, :], y_ps[:BP, :],
                                             ACT.Copy)
                        nc.sync.dma_start(
                            y_loc[li + 1][b * BP:b * BP + r, 0:D_next],
                            y_sb[:r, :])

                # Software pipeline: the one-hot for block b+1 is emitted
                # before block b's matmuls (DVE computes it while PE chews on
                # block b), and block b's epilogue tail is deferred behind
                # block b+1's matmuls so per-engine FIFO order never makes
                # DVE/ACT wait on PE round trips.
                def prefetch_gathers(b2):
                    if b2 >= nblk:
                        return
                    for h in (0, 1):
                        ensure_call(h, b2)

                oh_q = [emit_oh(i) for i in range(min(OH_DIST, nblk))]
                for i in range(min(PF_DIST, nblk)):
                    prefetch_gathers(i)
                pending = None
                for b in range(nblk):
                    if b + OH_DIST < nblk:
                        oh_q.append(emit_oh(b + OH_DIST))
                    prefetch_gathers(b + PF_DIST)
                    if pending is not None:
                        emit_tail(*pending)
                    oh_cur = oh_q.pop(0)
                    oh3 = oh_cur[:].rearrange("p (j k) -> p k j", k=CTOT)
                    ys = hpool.tile([P, D], ydt[li], tag="yself")
                    nc.sync.dma_start(ys[:rows_of[b], :],
                                      y_loc[li][b * BP:b * BP + rows_of[b],
                                                0:D])
                    agg = agg_pp.tile([P, D], F32, tag="agg")
                    for ci in range(CTOT):
                        h = 0 if ci < C_LO else 1
                        c = ci if ci < C_LO else ci - C_LO
                        g = ensure_call(h, b)
                        if SKIP_MM and ci > 0:
                            continue
                        nc.tensor.matmul(
                            agg[:BP, :], lhsT=oh3[:, ci, :],
                            rhs=g[:, c * EL:c * EL + D],
                            start=(ci == 0), stop=False)
                    r = rows_of[b]
                    nc.tensor.matmul(agg[:BP, :], lhsT=ident16[:r, :BP],
                                     rhs=ys[:r, :], start=False, stop=True)
                    pending = (b, agg)
                emit_tail(*pending)

            def prologue(warm=False):
                if warm:
                    # initialize every gather-tile buffer once so trailing
                    # (never-gathered) slots hold finite data
                    for h, nch in ((0, C_LO), (1, C_HI)):
                        for _ in range(GBUFS):
                            g = gpool.tile([P, nch * DH], FP16, tag=f"g{h}")
                            nc.vector.memset(g[:], 0)
                for b in range(NBLK):
                    y_ps = y_pp.tile([P, DH], F32, tag="yps")
                    nc.tensor.matmul(y_ps[:BP, :],
                                     lhsT=xT_sb[:, b * BP:(b + 1) * BP],
                                     rhs=w1t_sb[:], start=True, stop=True)
                    y_sb = hpool.tile([P, DH], ydt[0], tag="ysb")
                    nc.scalar.activation(y_sb[:BP, :], y_ps[:BP, :],
                                         ACT.Copy)
                    r = rows_of[b]
                    nc.sync.dma_start(y_loc[0][b * BP:b * BP + r, :],
                                      y_sb[:r, :])

            prologue(warm=True)
            stages = [
                lambda: allgather(0),
                lambda: agg_layer(0, DH, w2t_sb, DH),
                lambda: allgather(1),
                lambda: agg_layer(1, DH, w3t_sb, DZ),
                lambda: allgather(2),
                lambda: agg_layer(2, DZ, None, None),
            ]
            for st in stages[:BUILD_STAGES]:
                st()
            for _ in range(REPEATS - 1):
                prologue()
                for st in stages[:BUILD_STAGES]:
                    st()

    nc.compile()
    return nc


_cache = {}


def _get_nc(C_LO, C_HI):
    key = (C_LO, C_HI)
    if key not in _cache:
        _cache[key] = _build(C_LO, C_HI)
    return _cache[key]


def _in_maps(prep, W1, W2, W3):
    w1t = np.ascontiguousarray(np.asarray(W1, np.float32).T)
    w2t = np.ascontiguousarray(np.asarray(W2, np.float32).T)
    w3t = np.ascontiguousarray(np.asarray(W3, np.float32).T)
    return [{
        "xT": prep["xT"][c],
        "idx": prep["idx"][c],
        "dmod": prep["dmod"][c],
        "cnt": prep["cnt"][c],
        "iota": prep["iota"],
        "gcnt": prep["gcnt"][c][None, :],
        "w1t": w1t, "w2t": w2t, "w3t": w3t,
    } for c in range(NCORES)]


def kernel(x, edge_index, W1, W2, W3, _trace=False):
    prep = _preprocess(x, edge_index)
    nc = _get_nc(prep["C_LO"], prep["C_HI"])
    res = run_bass_kernel_spmd(nc, _in_maps(prep, W1, W2, W3),
                               list(range(NCORES)), trace=_trace)
    out = np.concatenate([res.results[c]["out"] for c in range(NCORES)],
                         axis=0).astype(np.float32)
    if _trace:
        kernel._last_results = res
    return out
